# revision 1
# baseline (speedup 1.0000x reference)
"""NT-Xent (SimCLR) contrastive loss on 8 Trainium2 NeuronCores.

Math: with z = concat(z_i, z_j) [2B, D], zn = z / ||z||_row,
logits = zn @ zn.T / T (diag masked), targets pair row r with r+-B.

loss = mean_r( LSE_r - l_r )
     = mean_r( log(S~_r) + C - 2*p_r )
where S~_r = sum_{c != r} exp(2*s_rc - C),  computed as
      S_r (full row sum incl. diag) - exp(2*d_r - C),
  s_rc = zn_r . zn_c,  d_r = zn_r . zn_r (self dot, ~1),
  p_r = zn_r . zn_partner(r),  C = 2.0 = 1/T (max possible logit).

Sharding: data-parallel over rows of the similarity matrix.  Every core
receives the FULL z, pre-rotated by 1024*k rows (host-side np.roll) so the
SPMD program always works on "rows 0..1023" — no per-core addressing.
Rotation is a permutation, so row sums over all 8192 columns are invariant,
and partner(r) = (r + 4096) % 8192 is rotation-invariant.

Device pipeline per core:
  1. gpsimd cast-DMA: z fp32 HBM -> bf16 SBUF natural chunks [128, 2048]
  2. per chunk: ss via tensor_tensor_reduce;  inv = Exp(-0.5 * Ln(ss)) (ACT,
     single exp/ln table set);  zn = z * inv (DVE broadcast);  16 DMA xbar
     transposes -> znT tiles [128, 2048] (columns = rotated rows)
  3. main loop over 4 column groups x 8 own row tiles: 8 bf16 matmuls
     (K = 2 x 128) -> PSUM [128, 2048];  ACT Exp(scale=2, bias=-2) with
     accum_out giving per-row partial sums
  4. pair/self dots via tensor_tensor_reduce on natural chunks 0 and 4
Host combines 8 cores' partial vectors: log, mean  (the all-reduce of the
sharding hint, done at unshard time).
"""

import numpy as np

import concourse.bacc as bacc
import concourse.mybir as mybir
import concourse.tile as tile
from concourse.bass_utils import run_bass_kernel_spmd

P = 128
D = 256
B = 4096
N2 = 2 * B            # 8192 rows total
NCORES = 8
NCH = 8               # row chunks of 1024
TPC = 8               # [128, D] tiles per chunk
CHW = TPC * D         # 2048: chunk width in natural layout
G = 4                 # column groups of 2048 in znT
M_TILES = 8           # own 1024 rows = 8 M-subtiles
TEMP = 0.5
CSTAB = 2.0           # stabilization constant = 1/T

f32 = mybir.dt.float32
bf16 = mybir.dt.bfloat16
AF = mybir.ActivationFunctionType
OP = mybir.AluOpType

import os
# Stage gate for HW bisection: 1=casts, 2=+ss/inv, 3=+normalize,
# 4=+transposes+d/p, 5=+main-loop-noaccum, 6=full (default)
STAGE = int(os.environ.get("KERNEL_STAGE", "6"))


def _emit(tc, z, s_out, d_out, p_out):
    nc = tc.nc

    with tc.tile_pool(name="raw", bufs=NCH) as raw_pool, \
            tc.tile_pool(name="zn", bufs=NCH) as zn_pool, \
            tc.tile_pool(name="znt", bufs=2 * G) as znt_pool, \
            tc.tile_pool(name="small", bufs=3) as small_pool, \
            tc.tile_pool(name="ttrash", bufs=3) as ttrash_pool, \
            tc.tile_pool(name="etrash", bufs=2) as etrash_pool, \
            tc.tile_pool(name="acc", bufs=1) as acc_pool, \
            tc.tile_pool(name="dram", bufs=NCH, space="DRAM") as dram_pool, \
            tc.tile_pool(name="psum", bufs=2, space="PSUM") as psum_pool:
        # znT[h][g]: columns 2048*g .. of the transposed normalized z for
        # d-half h.  8 tiles, all live for the whole kernel.
        znt = [[znt_pool.tile([P, CHW], bf16, tag="znt", name=f"znt{h}_{g}") for g in range(G)]
               for h in range(2)]

        rs_buf = acc_pool.tile([P, M_TILES * G], f32, tag="rs", name="rs_buf")
        bias_t = acc_pool.tile([P, 1], f32, tag="bias", name="bias_t")
        nc.vector.memset(bias_t[:], -CSTAB)
        ss_all = acc_pool.tile([P, NCH * TPC], f32, tag="ssall", name="ss_all")
        inv_all = acc_pool.tile([P, NCH * TPC], bf16, tag="invall", name="inv_all")
        s_sb = acc_pool.tile([P, M_TILES], f32, tag="ssb", name="s_sb")
        d_sb = acc_pool.tile([P, TPC], f32, tag="dsb", name="d_sb")
        p_sb = acc_pool.tile([P, TPC], f32, tag="psb", name="p_sb")

        for t in (s_sb, d_sb, p_sb):
            nc.vector.memset(t[:], 1.0)
        nc.vector.memset(rs_buf[:], 1.0)

        raws = []
        for c in range(NCH):
            raw = raw_pool.tile([P, CHW], bf16, tag="raw", name=f"raw{c}")
            src = z[1024 * c:1024 * (c + 1), :].rearrange(
                "(n p) d -> p n d", p=P)
            nc.gpsimd.dma_start(
                out=raw[:].rearrange("p (n d) -> p n d", d=D), in_=src)
            raws.append(raw)
            if STAGE < 2:
                continue
            sq = ttrash_pool.tile([P, CHW], bf16, tag="tt", name="tt")
            nc.vector.tensor_tensor(sq[:], raw[:], raw[:], op=OP.mult)
            nc.vector.reduce_sum(
                out=ss_all[:, TPC * c:TPC * (c + 1)].unsqueeze(-1),
                in_=sq[:].rearrange("p (t d) -> p t d", t=TPC),
                axis=mybir.AxisListType.X)

        def lnexp(lo, hi):
            # inv = Exp(-0.5*Ln(ss)) for norm columns [lo, hi)
            ln_t = small_pool.tile([P, hi - lo], f32, tag="lnall",
                                   name="ln_t")
            nc.scalar.activation(ln_t[:], ss_all[:, lo:hi], AF.Ln)
            nc.scalar.activation(inv_all[:, lo:hi], ln_t[:], AF.Exp,
                                 scale=-0.5)

        zns = []

        def norm_transpose(c):
            zn = zn_pool.tile([P, CHW], bf16, tag="zn", name=f"zn{c}")
            nc.vector.tensor_tensor(
                out=zn[:].rearrange("p (t d) -> p t d", t=TPC),
                in0=raws[c][:].rearrange("p (t d) -> p t d", t=TPC),
                in1=inv_all[:, TPC * c:TPC * (c + 1)].unsqueeze(-1)
                    .broadcast_to([P, TPC, D]),
                op=OP.mult)
            zns.append(zn)
            if STAGE < 4:
                return
            zb = dram_pool.tile([1024, D], bf16, tag="zb", name=f"zb{c}")
            nc.sync.dma_start(
                out=zb[:].rearrange("(t p) d -> p t d", p=P),
                in_=zn[:].rearrange("p (t d) -> p t d", t=TPC))
            g = c // 2
            off = 1024 * (c % 2)
            for h in range(2):
                nc.scalar.dma_start(
                    out=znt[h][g][:, off:off + 1024],
                    in_=zb[:, 128 * h:128 * (h + 1)],
                    transpose=True)

        # Two-phase inv: chunks 0,1 feed every lhsT and column-group 0,
        # so finish their whole chain first and let PE start early.
        if STAGE >= 2:
            lnexp(0, 2 * TPC)
        if STAGE >= 3:
            for c in (0, 1):
                norm_transpose(c)
        if STAGE >= 2:
            lnexp(2 * TPC, NCH * TPC)
        if STAGE >= 3:
            for c in range(2, NCH):
                norm_transpose(c)

        # Self dots (match the matmul's bf16 diagonal) and pair dots.
        if STAGE >= 4:
            for src0, src1, dst in ((zns[0], zns[0], d_sb),
                                    (zns[0], zns[4], p_sb)):
                tt = ttrash_pool.tile([P, CHW], bf16, tag="tt", name="tt")
                nc.vector.tensor_tensor(tt[:], src0[:], src1[:], op=OP.mult)
                nc.vector.reduce_sum(
                    out=dst[:].unsqueeze(-1),
                    in_=tt[:].rearrange("p (t d) -> p t d", t=TPC),
                    axis=mybir.AxisListType.X)

        # Main loop: rows 0..1023 (own) x all 8192 columns.
        if STAGE >= 5:
            for g in range(G):
                for m in range(M_TILES):
                    ps = psum_pool.tile([P, CHW], f32, tag="ps",
                                        name=f"ps{g}_{m}")
                    for h in range(2):
                        for c4 in range(4):
                            nc.tensor.matmul(
                                out=ps[:, 512 * c4:512 * (c4 + 1)],
                                lhsT=znt[h][0][:, 128 * m:128 * (m + 1)],
                                rhs=znt[h][g][:, 512 * c4:512 * (c4 + 1)],
                                start=(h == 0), stop=(h == 1))
                    et = etrash_pool.tile([P, CHW], bf16, tag="et",
                                          name=f"et{g}_{m}")
                    idx = 4 * m + g
                    if STAGE == 5:
                        nc.scalar.activation(
                            et[:], ps[:], AF.Exp, bias=bias_t[:],
                            scale=1.0 / TEMP)
                    else:
                        nc.scalar.activation(
                            et[:], ps[:], AF.Exp, bias=bias_t[:],
                            scale=1.0 / TEMP,
                            accum_out=rs_buf[:, idx:idx + 1])

        nc.vector.reduce_sum(
            out=s_sb[:].unsqueeze(-1),
            in_=rs_buf[:].rearrange("p (m g) -> p m g", g=G),
            axis=mybir.AxisListType.X)

        nc.sync.dma_start(out=s_out, in_=s_sb[:])
        nc.sync.dma_start(out=d_out, in_=d_sb[:])
        nc.sync.dma_start(out=p_out, in_=p_sb[:])


def build():
    nc = bacc.Bacc("TRN2", target_bir_lowering=False, debug=False)
    z = nc.dram_tensor("z", [N2, D], f32, kind="ExternalInput").ap()
    s_out = nc.dram_tensor("s_out", [P, M_TILES], f32, kind="ExternalOutput").ap()
    d_out = nc.dram_tensor("d_out", [P, TPC], f32, kind="ExternalOutput").ap()
    p_out = nc.dram_tensor("p_out", [P, TPC], f32, kind="ExternalOutput").ap()
    with tile.TileContext(nc) as tc:
        _emit(tc, z, s_out, d_out, p_out)
    nc.compile()
    return nc


def make_in_maps(z_i, z_j):
    z_full = np.concatenate(
        [np.asarray(z_i, dtype=np.float32), np.asarray(z_j, dtype=np.float32)],
        axis=0)
    return [{"z": np.ascontiguousarray(np.roll(z_full, -1024 * k, axis=0))}
            for k in range(NCORES)]


def combine(results):
    S = np.empty(N2, np.float64)
    dv = np.empty(N2, np.float64)
    pv = np.empty(N2, np.float64)
    pp = np.arange(P)[:, None]
    mm = np.arange(M_TILES)[None, :]
    for k in range(NCORES):
        gidx = ((1024 * k + 128 * mm + pp) % N2).ravel()
        S[gidx] = results[k]["s_out"].astype(np.float64).ravel()
        dv[gidx] = results[k]["d_out"].astype(np.float64).ravel()
        pv[gidx] = results[k]["p_out"].astype(np.float64).ravel()
    St = S - np.exp(dv / TEMP - CSTAB)
    lse = np.log(St) + CSTAB
    loss = np.mean(lse - pv / TEMP)
    return np.asarray(loss, dtype=np.float32)


_NC_CACHE = None


def kernel(z_i, z_j):
    global _NC_CACHE
    if _NC_CACHE is None:
        _NC_CACHE = build()
    res = run_bass_kernel_spmd(
        _NC_CACHE, make_in_maps(z_i, z_j), list(range(NCORES))).results
    return combine(res)



# revision 5
# speedup vs baseline: 1.3782x; 1.3782x over previous
"""NT-Xent (SimCLR) contrastive loss on 8 Trainium2 NeuronCores — v2.

Math: z = concat(z_i, z_j) [2B, D], zn = z / ||z||_row,
logits = zn @ zn.T / T (diag masked), targets pair row r with r±B.
loss = mean_r(LSE_r - 2*p_r),  LSE_r = log(S_r - diag_r) + 2,
  S_r = sum_c exp(2*s_rc - 2),  p_r = zn_r . zn_partner(r).

Sharding: data-parallel rows. Core k gets z pre-rotated by 1024k rows
(host np.roll), computes its 1024 rows x all 8192 cols.

v2 pipeline (per core):
  Host feeds bf16 natural z AND a bf16 transposed copy whose columns are
  permuted per-1024-chunk as i = 8q + t (q=0..127, t=0..7, row=128t+q).
  That makes the device-computed inv vector land LINEARLY in DRAM so it
  can be broadcast-read back across partitions with unit-stride DMA.
  Per chunk c: ss (DVE square+reduce) -> inv16 = exp(-ln(ss)/2 + ln16)
  (ACT, shares the ln/exp table set with the main loop) -> DRAM write +
  partition-replicated read -> normalize transposed chunk to fp8 e4m3
  (x16 scale) -> fp8 DoubleRow matmuls (full K=256 per instr, 0.5
  cyc/col) -> ACT exp(psum*2/256 - 2) with accum_out row sums.
  Pair logits from bf16 raw dots (rows 0..1023 x 4096..5119); host
  divides by fp64 norms. Diagonal approximated as exp(0)=1 at combine
  (error ~1e-4 of the row sum).
Host combine = the all-reduce: assemble S, subtract diag, log, mean.
"""

import math

import numpy as np
import ml_dtypes

import concourse.bacc as bacc
import concourse.mybir as mybir
import concourse.tile as tile
from concourse.bass_utils import run_bass_kernel_spmd

P = 128
D = 256
B = 4096
N2 = 2 * B            # 8192 rows
NCORES = 8
NCH = 8               # 1024-row/col chunks
CW = 1024             # chunk width
TPC = 8               # [128, D] row-tiles per natural chunk
G = 4                 # psum column groups of 2048
M_TILES = 8
TEMP = 0.5
CSTAB = 2.0
SCALE_EXP = 2.0 / 256.0   # psum holds 256*s
LN16 = math.log(16.0)

f32 = mybir.dt.float32
bf16 = mybir.dt.bfloat16
fp8 = mybir.dt.float8e4
AF = mybir.ActivationFunctionType
OP = mybir.AluOpType
DR = mybir.MatmulPerfMode.DoubleRow


def _emit(tc, znat_d, zt_d, s_out, rawp_out):
    nc = tc.nc

    with tc.tile_pool(name="nat", bufs=NCH) as nat_pool, \
            tc.tile_pool(name="zth", bufs=2 * NCH) as zth_pool, \
            tc.tile_pool(name="zn8", bufs=NCH) as zn8_pool, \
            tc.tile_pool(name="invb", bufs=NCH) as invb_pool, \
            tc.tile_pool(name="sqt", bufs=2) as sq_pool, \
            tc.tile_pool(name="ett", bufs=2) as et_pool, \
            tc.tile_pool(name="small", bufs=4 * NCH + 8) as small_pool, \
            tc.tile_pool(name="acc", bufs=4) as acc_pool, \
            tc.tile_pool(name="dram", bufs=2, space="DRAM") as dram_pool, \
            tc.tile_pool(name="psum", bufs=2, space="PSUM") as psum_pool:

        bias_t = acc_pool.tile([P, 1], f32, tag="bias", name="bias_t")
        nc.vector.memset(bias_t[:], -CSTAB)
        ln16_t = acc_pool.tile([P, 1], f32, tag="ln16", name="ln16_t")
        nc.vector.memset(ln16_t[:], LN16)
        rs = acc_pool.tile([P, M_TILES * G], f32, tag="rs", name="rs")
        s_sb = acc_pool.tile([P, M_TILES], f32, tag="ssb", name="s_sb")
        rawp = acc_pool.tile([P, TPC], f32, tag="rawp", name="rawp")

        inv_d = dram_pool.tile([N2], bf16, tag="invd", name="inv_d")

        nats, zths, zn8s, invbs = [], [], [], []
        for c in range(NCH):
            # natural chunk: rows 1024c..1024c+1024 as [128, 8, 256]
            nat = nat_pool.tile([P, TPC * D], bf16, tag="nat", name=f"nat{c}")
            nc.sync.dma_start(
                out=nat[:].rearrange("p (n d) -> p n d", d=D),
                in_=znat_d[CW * c:CW * (c + 1), :].rearrange(
                    "(n p) d -> p n d", p=P))
            nats.append(nat)
            # transposed chunk halves: [128 d, 1024 cols] each
            hh = []
            for h in range(2):
                zth = zth_pool.tile([P, CW], bf16, tag="zth",
                                    name=f"zth{c}_{h}")
                eng = nc.sync if h == 0 else nc.gpsimd
                eng.dma_start(out=zth[:],
                              in_=zt_d[P * h:P * (h + 1),
                                       CW * c:CW * (c + 1)])
                hh.append(zth)
            zths.append(hh)

        for c in range(NCH):
            nat = nats[c]
            # ss for this chunk's 1024 rows -> [128, 8]
            sq = sq_pool.tile([P, TPC * D], bf16, tag="sq", name="sq")
            nc.vector.tensor_tensor(sq[:], nat[:], nat[:], op=OP.mult)
            ss_c = small_pool.tile([P, TPC], f32, tag="ss", name=f"ss{c}")
            nc.vector.reduce_sum(
                out=ss_c[:].unsqueeze(-1),
                in_=sq[:].rearrange("p (t d) -> p t d", t=TPC),
                axis=mybir.AxisListType.X)
            # inv16 = exp(-0.5 ln ss + ln 16)  (one ln/exp table set)
            ln_c = small_pool.tile([P, TPC], f32, tag="ln", name=f"ln{c}")
            nc.scalar.activation(ln_c[:], ss_c[:], AF.Ln)
            iv_c = small_pool.tile([P, TPC], bf16, tag="iv", name=f"iv{c}")
            nc.scalar.activation(iv_c[:], ln_c[:], AF.Exp,
                                 scale=-0.5, bias=ln16_t[:])
            # DRAM roundtrip: linear write (i = 8p + t), replicated read
            nc.gpsimd.dma_start(
                out=inv_d[CW * c:CW * (c + 1)].rearrange("(p t) -> p t",
                                                         t=TPC),
                in_=iv_c[:])
            invb = invb_pool.tile([P, CW], bf16, tag="invb", name=f"invb{c}")
            nc.scalar.dma_start(
                out=invb[:],
                in_=inv_d[CW * c:CW * (c + 1)].unsqueeze(0)
                    .broadcast_to([P, CW]))
            invbs.append(invb)
            # normalize transposed chunk -> fp8 (x16 via inv16)
            zn8 = zn8_pool.tile([P, 2 * CW], fp8, tag="zn8", name=f"zn8{c}")
            for h in range(2):
                nc.vector.tensor_tensor(
                    zn8[:, CW * h:CW * (h + 1)], zths[c][h][:], invb[:],
                    op=OP.mult)
            zn8s.append(zn8)

        # raw pair dots: rolled rows 0..1023 (chunk 0) x 4096..5119 (chunk 4)
        tt = sq_pool.tile([P, TPC * D], bf16, tag="sq", name="tt")
        nc.vector.tensor_tensor(tt[:], nats[0][:], nats[4][:], op=OP.mult)
        nc.vector.reduce_sum(
            out=rawp[:].unsqueeze(-1),
            in_=tt[:].rearrange("p (t d) -> p t d", t=TPC),
            axis=mybir.AxisListType.X)
        nc.sync.dma_start(out=rawp_out, in_=rawp[:])

        # main loop: 1024 own rows x 8192 cols, fp8 DoubleRow (K=256/instr)
        def lhsT(m):
            # own rows: chunk-0 cols i = 8q + m  ->  [128, 2, 128]
            return zn8s[0][:].rearrange("p (h q t) -> p h t q", h=2,
                                        t=TPC)[:, :, m, :]

        def rhs(chunk, off):
            return zn8s[chunk][:].rearrange(
                "p (h w) -> p h w", h=2)[:, :, off:off + 512]

        for g in range(G):
            for m in range(M_TILES):
                ps = psum_pool.tile([P, 2048], f32, tag="ps",
                                    name=f"ps{g}_{m}")
                for c4 in range(4):
                    nc.tensor.matmul(
                        out=ps[:, 512 * c4:512 * (c4 + 1)],
                        lhsT=lhsT(m),
                        rhs=rhs(2 * g + c4 // 2, 512 * (c4 % 2)),
                        start=True, stop=True, perf_mode=DR)
                et = et_pool.tile([P, 2048], bf16, tag="et", name=f"et{g}_{m}")
                idx = 4 * m + g
                nc.scalar.activation(et[:], ps[:], AF.Exp, bias=bias_t[:],
                                     scale=SCALE_EXP,
                                     accum_out=rs[:, idx:idx + 1])

        nc.vector.reduce_sum(
            out=s_sb[:].unsqueeze(-1),
            in_=rs[:].rearrange("p (m g) -> p m g", g=G),
            axis=mybir.AxisListType.X)
        nc.sync.dma_start(out=s_out, in_=s_sb[:])


def build():
    nc = bacc.Bacc("TRN2", target_bir_lowering=False, debug=False)
    znat = nc.dram_tensor("znat", [N2, D], bf16, kind="ExternalInput").ap()
    zt = nc.dram_tensor("zt", [D, N2], bf16, kind="ExternalInput").ap()
    s_out = nc.dram_tensor("s_out", [P, M_TILES], f32,
                           kind="ExternalOutput").ap()
    rawp_out = nc.dram_tensor("rawp_out", [P, TPC], f32,
                              kind="ExternalOutput").ap()
    with tile.TileContext(nc) as tc:
        _emit(tc, znat, zt, s_out, rawp_out)
    nc.compile()
    return nc


_COLMAP = None


def _colmap():
    global _COLMAP
    if _COLMAP is None:
        i = np.arange(N2)
        _COLMAP = (i // CW) * CW + (i % TPC) * P + (i % CW) // TPC
    return _COLMAP


def make_in_maps(z_i, z_j):
    z_full = np.concatenate(
        [np.asarray(z_i, dtype=np.float32), np.asarray(z_j, dtype=np.float32)],
        axis=0)
    rmap = _colmap()
    maps = []
    for k in range(NCORES):
        zr = np.roll(z_full, -CW * k, axis=0)
        znat = np.ascontiguousarray(zr).astype(ml_dtypes.bfloat16)
        zt = np.ascontiguousarray(zr.T[:, rmap]).astype(ml_dtypes.bfloat16)
        maps.append({"znat": znat, "zt": zt})
    return maps


def combine(results, z_full):
    n = np.linalg.norm(z_full.astype(np.float64), axis=1)
    S = np.empty(N2, np.float64)
    pv = np.empty(N2, np.float64)
    pp = np.arange(P)[:, None]
    mm = np.arange(M_TILES)[None, :]
    for k in range(NCORES):
        gidx = ((CW * k + P * mm + pp) % N2).ravel()
        S[gidx] = results[k]["s_out"].astype(np.float64).ravel()
        pv[gidx] = results[k]["rawp_out"].astype(np.float64).ravel()
    St = S - 1.0                       # drop diag (exp(2*d-2) ~= 1)
    lse = np.log(St) + CSTAB
    partner = (np.arange(N2) + B) % N2
    p = pv / (n * n[partner])
    loss = np.mean(lse - 2.0 * p)
    return np.asarray(loss, dtype=np.float32)


_NC_CACHE = None


def kernel(z_i, z_j):
    global _NC_CACHE
    if _NC_CACHE is None:
        _NC_CACHE = build()
    z_full = np.concatenate(
        [np.asarray(z_i, dtype=np.float32), np.asarray(z_j, dtype=np.float32)],
        axis=0)
    res = run_bass_kernel_spmd(
        _NC_CACHE, make_in_maps(z_i, z_j), list(range(NCORES))).results
    return combine(res, z_full)
